# revision 1
# baseline (speedup 1.0000x reference)
"""CfC recurrence kernel for Trainium2, 8 NeuronCores.

Sharding: data-parallel over batch B=8 (one sample per core); W_f/W_g/W_proj
replicated. The sequential T=2048 recurrence is fully unrolled (the `loop`
builder path exists but register-offset APs capture trace-time values, so the
shipped kernel uses loop=False).

Per-core algorithm (sample s):
  phase 0: load weights (f32) -> bf16 SBUF tiles
  phase 1: precompute A[t, :] = x_t @ [W_fx | 2*W_gx]   (parallel over t)
  phase 2: sequential scan: z_t = A[t] + 0.5*[W_fh | 2*W_gh]^T (2 h_{t-1})
           u = tanh(0.5 z) ;  f = 0.5 u_f + 0.5, g = u_g
           h_t = 0.5 [ u_f (h-g) + h + g ]
           (the broadcast state is 2h in bf16 -- the p+q add writes it
            directly, W_fh/W_gh are pre-halved on the host to compensate)
  phase 3: y = H @ W_proj  (parallel over t)

The sigmoid is computed via sigmoid(z) = 0.5 tanh(z/2) + 0.5 and the g-gate
weights are pre-doubled on the host so one Tanh activation (scale=0.5) covers
both gates.

Layouts (per core):
  xt     DRAM [1024, 2048] f32  = x[s].T
  wh     DRAM [1024, 2048] f32  = 0.5*[W_f[C:2C, :] | 2*W_g[C:2C, :]]
  wx     DRAM [1024, 2048] f32  = [W_f[0:C, :]  | 2*W_g[0:C, :]]
  wproj  DRAM [1024, 1024] f32
  y      DRAM [1024, 2048] f32  = (output)[c_out, t]; host transposes.
"""

import sys

for _p in ("/opt/trn_rl_repo", "/root/.axon_site/_ro/trn_rl_repo"):
    if _p not in sys.path:
        sys.path.insert(0, _p)

import numpy as np

from concourse import bass, bacc, bass_utils
import concourse.mybir as mybir

B, T, C = 8, 2048, 1024
K = 8          # c_in chunks of 128
MT = 16        # gate output tiles of 128 (8 f + 8 g)
PT = 8         # projection output tiles
NG = T // 512  # 512-row groups for the parallel matmul phases
STEPS_PER_ITER = 64
FORCE_CONST_WAITS = False
FORCE_CONST_APS = False
F32 = mybir.dt.float32
BF16 = mybir.dt.bfloat16


def build_nc(t_total=T, loop=True):
    ng = t_total // 512
    nit = t_total // STEPS_PER_ITER

    nc = bacc.Bacc("TRN2", target_bir_lowering=False, debug=False)

    xt = nc.dram_tensor("xt", [C, t_total], F32, kind="ExternalInput")
    wh = nc.dram_tensor("wh", [C, 2 * C], F32, kind="ExternalInput")
    wx = nc.dram_tensor("wx", [C, 2 * C], F32, kind="ExternalInput")
    wp = nc.dram_tensor("wp", [C, C], F32, kind="ExternalInput")
    y = nc.dram_tensor("y", [C, t_total], F32, kind="ExternalOutput")

    # SBUF (bytes/partition):
    whs = nc.alloc_sbuf_tensor("whs", [128, K * 2 * C], BF16)      # 32KB/p
    wxs = nc.alloc_sbuf_tensor("wxs", [128, K * 2 * C], BF16)      # 32KB/p (reused as hist in phase 2+)
    wps = nc.alloc_sbuf_tensor("wps", [128, K * C], BF16)          # 16KB/p
    a_sb = nc.alloc_sbuf_tensor("a_sb", [128, t_total * MT], BF16)  # 64KB/p
    stag = nc.alloc_sbuf_tensor("stag", [128, 4096], F32)          # 16KB/p
    xbf = nc.alloc_sbuf_tensor("xbf", [128, K * 512], BF16)        # 8KB/p
    h32 = nc.alloc_sbuf_tensor("h32", [128, 8], F32)
    hbf = nc.alloc_sbuf_tensor("hbf", [128, 8], BF16)
    za_sb = nc.alloc_sbuf_tensor("za_sb", [128, 32], F32)  # 2 slots of 16
    u_sb = nc.alloc_sbuf_tensor("u_sb", [128, 16], F32)
    d_sb = nc.alloc_sbuf_tensor("d_sb", [128, 8], F32)
    q_sb = nc.alloc_sbuf_tensor("q_sb", [128, 8], F32)
    p_sb = nc.alloc_sbuf_tensor("p_sb", [128, 8], F32)
    r_sb = nc.alloc_sbuf_tensor("r_sb", [128, 8], F32)
    ysb0 = nc.alloc_sbuf_tensor("ysb0", [128, 512], F32)
    ysb1 = nc.alloc_sbuf_tensor("ysb1", [128, 512], F32)
    ysb = [ysb0, ysb1]

    zps = nc.alloc_psum_tensor("zps", [128, 16], F32)
    ppre0 = nc.alloc_psum_tensor("ppre0", [128, 512], F32)
    ppre1 = nc.alloc_psum_tensor("ppre1", [128, 512], F32)
    ppre = [ppre0, ppre1]
    pproj0 = nc.alloc_psum_tensor("pproj0", [128, 512], F32)
    pproj1 = nc.alloc_psum_tensor("pproj1", [128, 512], F32)
    pproj = [pproj0, pproj1]

    s_dw0 = nc.alloc_semaphore("s_dw0")
    s_dw1 = nc.alloc_semaphore("s_dw1")
    s_dw = [s_dw0, s_dw1]
    s_dmax = nc.alloc_semaphore("s_dmax")
    s_conv = nc.alloc_semaphore("s_conv")
    s_zpre = nc.alloc_semaphore("s_zpre")
    s_pre = nc.alloc_semaphore("s_pre")
    s_z = nc.alloc_semaphore("s_z")
    s_za = nc.alloc_semaphore("s_za")
    s_u = nc.alloc_semaphore("s_u")
    s_uf = nc.alloc_semaphore("s_uf")
    s_h = nc.alloc_semaphore("s_h")
    s_zproj = nc.alloc_semaphore("s_zproj")
    s_c1 = nc.alloc_semaphore("s_c1")
    s_c2 = nc.alloc_semaphore("s_c2")
    s_c3 = nc.alloc_semaphore("s_c3")
    s_c4 = nc.alloc_semaphore("s_c4")
    s_hist = nc.alloc_semaphore("s_hist")
    s_proj = nc.alloc_semaphore("s_proj")
    s_out0 = nc.alloc_semaphore("s_out0")
    s_out1 = nc.alloc_semaphore("s_out1")
    s_out = [s_out0, s_out1]

    # phase-0 load descriptors: (src_ap, conv_dst_ap, width)
    loads = []
    for k in range(K):
        loads.append((wh[k * 128:(k + 1) * 128, :],
                      whs[:, k * 2048:(k + 1) * 2048], 2048))
    for k in range(K):
        loads.append((wx[k * 128:(k + 1) * 128, :],
                      wxs[:, k * 2048:(k + 1) * 2048], 2048))
    for k in range(K):
        loads.append((wp[k * 128:(k + 1) * 128, :],
                      wps[:, k * 1024:(k + 1) * 1024], 1024))
    n_loads = len(loads)  # 24

    def whs_tile(k, m):
        off = (k * MT + m) * 128
        return whs[:, off:off + 128]

    def wxs_tile(k, m):
        off = (k * MT + m) * 128
        return wxs[:, off:off + 128]

    def wps_tile(k, m):
        off = (k * PT + m) * 128
        return wps[:, off:off + 128]

    # hist aliases wxs: [128, chunk(8), t] bf16 (chunk-major)
    hist_r = wxs.ap().rearrange("p (c t) -> p c t", c=K)
    a_r = a_sb.ap().rearrange("p (t m) -> p t m", m=MT)
    xbf_r = xbf.ap().rearrange("p (c t) -> p c t", c=K)
    stag_x = stag.ap().rearrange("p (c t) -> p c t", c=K)
    hbf_3 = hbf.ap().rearrange("p (c o) -> p c o", o=1)

    with nc.Block() as block:

        @block.sync
        def _(sync):
            mainbb = nc.cur_bb
            from contextlib import nullcontext
            if loop:
                sync.br("sy_p0")
            with (nc.bb("sy_p0", parent=mainbb) if loop else nullcontext()):
                for i, (src, _dst, _w) in enumerate(loads):
                    if i >= 2:
                        sync.wait_ge(s_conv, i - 1)
                    half = stag[:, (i % 2) * 2048:(i % 2) * 2048 + loads[i][2]]
                    sync.dma_start(half, src).then_inc(s_dw[i % 2], 16)
                for g in range(ng):
                    sync.wait_ge(s_conv, n_loads + g)
                    sync.dma_start(
                        stag_x[:, :, :512],
                        xt[:, g * 512:(g + 1) * 512].rearrange(
                            "(c p) t -> p c t", p=128),
                    ).then_inc(s_dmax, 16)
                if loop:
                    sync.br("sy_p3")
            with (nc.bb("sy_p3", parent=mainbb) if loop else nullcontext()):
                for idx in range(PT * ng):
                    m, g = idx // ng, idx % ng
                    sync.wait_ge(s_proj, idx + 1)
                    sync.dma_start(
                        y[m * 128:(m + 1) * 128, g * 512:(g + 1) * 512],
                        ysb[idx % 2][:],
                    ).then_inc(s_out[idx % 2], 16)
                sync.wait_ge(s_out[0], 16 * ((PT * ng + 1) // 2))
                sync.wait_ge(s_out[1], 16 * (PT * ng // 2))
                if loop:
                    sync.br(block.end_bb)

        @block.vector
        def _(vector):
            mainbb = nc.cur_bb
            from contextlib import nullcontext
            if loop:
                vector.br("ve_p01")
            with vector.register("dve_cnt") as dve_cnt, \
                 vector.register("a_off") as a_off, \
                 vector.register("t_off") as t_off, \
                 vector.register("u_cnt") as u_cnt, \
                 vector.register("jv") as jv:
                with (nc.bb("ve_p01", parent=mainbb) if loop else nullcontext()):
                    # phase 0 conversions
                    for i, (_src, dst, w) in enumerate(loads):
                        vector.wait_ge(s_dw[i % 2], 16 * (i // 2 + 1))
                        half = stag[:, (i % 2) * 2048:(i % 2) * 2048 + w]
                        vector.tensor_copy(dst, half).then_inc(s_conv, 1)
                    # phase 1 x conversions
                    for g in range(ng):
                        vector.wait_ge(s_dmax, 16 * (g + 1))
                        if g >= 1:
                            vector.wait_ge(s_zpre, MT * g)
                        vector.tensor_copy(xbf[:], stag[:]).then_inc(s_conv, 1)
                    # phase 2 preamble
                    vector.memset(h32[:], 0.0)
                    vector.memset(hbf[:], 0.0).then_inc(s_h, 1)
                    vector.reg_mov(dve_cnt, 1)
                    vector.reg_mov(u_cnt, 0)
                    vector.reg_mov(a_off, 0)
                    vector.reg_mov(t_off, 0)
                    vector.reg_mov(jv, 0)
                    if loop:
                        vector.br("ve_chk")
                sv_a = vector.snap(a_off, donate=True)
                sv_t = vector.snap(t_off, donate=True)
                from contextlib import nullcontext
                if loop:
                    cm1 = nc.bb("ve_chk", parent=mainbb)
                else:
                    cm1 = nullcontext()
                with cm1:
                    if loop:
                        vector.br_lt(jv, nit, "ve_body", "ve_end")
                with (nc.bb("ve_body", parent=mainbb) if loop else nullcontext()):
                    n_inner = STEPS_PER_ITER if loop else t_total
                    r_3 = r_sb.ap().rearrange("p (c o) -> p c o", o=1)
                    for i in range(n_inner):
                        cnt = (i + 1) if (not loop or FORCE_CONST_WAITS) else dve_cnt
                        ucv = i if (not loop or FORCE_CONST_WAITS) else u_cnt
                        za_slot = za_sb[:, (i % 2) * 16:(i % 2) * 16 + 16]
                        vector.wait_ge(s_z, cnt)
                        a_ap = (a_sb[:, i * 16:(i + 1) * 16]
                                if (not loop or FORCE_CONST_APS)
                                else a_sb[:, bass.DynSlice(sv_a, 16)])
                        vector.tensor_add(
                            za_slot, zps[:], a_ap,
                        ).then_inc(s_za, 1)
                        vector.wait_ge(s_u, cnt)
                        vector.wait_ge(s_hist, ucv)  # hist(t-1) done reading r_sb
                        uf, ug = u_sb[:, 0:8], u_sb[:, 8:16]
                        vector.tensor_sub(d_sb[:], h32[:], ug).then_inc(s_c1, 1)
                        vector.tensor_add(q_sb[:], h32[:], ug).then_inc(s_c2, 1)
                        vector.wait_ge(s_c1, cnt)
                        vector.tensor_mul(p_sb[:], uf, d_sb[:]).then_inc(s_c3, 1)
                        vector.wait_ge(s_c2, cnt)
                        vector.wait_ge(s_c3, cnt)
                        # hbf = p + q = 2*h in bf16; W_h/W_proj are pre-halved
                        # on the host so downstream matmuls see h exactly.
                        vector.tensor_add(hbf[:], p_sb[:], q_sb[:]).then_inc(s_h, 1)
                        vector.tensor_add(r_sb[:], p_sb[:], q_sb[:]).then_inc(s_c4, 1)
                        vector.wait_ge(s_c4, cnt)
                        vector.tensor_scalar_mul(h32[:], r_sb[:], 0.5)
                        h_ap = (hist_r[:, :, i:i + 1]
                                if (not loop or FORCE_CONST_APS)
                                else hist_r[:, :, bass.DynSlice(sv_t, 1)])
                        vector.wait_ge(s_c4, cnt)
                        vector.tensor_scalar_mul(h_ap, r_3, 0.5).then_inc(s_hist, 1)
                        if loop:
                            vector.reg_add(dve_cnt, dve_cnt, 1)
                            vector.reg_add(u_cnt, u_cnt, 1)
                            vector.reg_add(a_off, a_off, 16)
                            vector.reg_add(t_off, t_off, 1)
                    if loop:
                        vector.reg_add(jv, jv, 1)
                        vector.br("ve_chk")
                if loop:
                    with nc.bb("ve_end", parent=mainbb):
                        vector.br(block.end_bb)

        @block.scalar
        def _(scalar):
            mainbb = nc.cur_bb
            from contextlib import nullcontext
            if loop:
                scalar.br("sc_p1")
            with scalar.register("act_cnt") as act_cnt, \
                 scalar.register("ja") as ja:
                with (nc.bb("sc_p1", parent=mainbb) if loop else nullcontext()):
                    for idx in range(ng * MT):
                        g, m = idx // MT, idx % MT
                        scalar.wait_ge(s_zpre, idx + 1)
                        scalar.copy(
                            a_r[:, g * 512:(g + 1) * 512, m],
                            ppre[idx % 2][:],
                        ).then_inc(s_pre, 1)
                    scalar.reg_mov(act_cnt, 1)
                    scalar.reg_mov(ja, 0)
                    if loop:
                        scalar.br("sc_chk")
                from contextlib import nullcontext
                with (nc.bb("sc_chk", parent=mainbb) if loop else nullcontext()):
                    if loop:
                        scalar.br_lt(ja, nit, "sc_body", "sc_p3")
                with (nc.bb("sc_body", parent=mainbb) if loop else nullcontext()):
                    for i in range(STEPS_PER_ITER if loop else t_total):
                        scalar.wait_ge(s_za, (i + 1) if (not loop or FORCE_CONST_WAITS) else act_cnt)
                        zbase = (i % 2) * 16
                        scalar.activation(
                            u_sb[:], za_sb[:, zbase:zbase + 16],
                            mybir.ActivationFunctionType.Tanh, scale=0.5,
                        ).then_inc(s_u, 1)
                        if loop:
                            scalar.reg_add(act_cnt, act_cnt, 1)
                    if loop:
                        scalar.reg_add(ja, ja, 1)
                        scalar.br("sc_chk")
                with (nc.bb("sc_p3", parent=mainbb) if loop else nullcontext()):
                    for idx in range(PT * ng):
                        scalar.wait_ge(s_zproj, idx + 1)
                        if idx >= 2:
                            scalar.wait_ge(s_out[idx % 2], 16 * ((idx - 2) // 2 + 1))
                        scalar.copy(ysb[idx % 2][:], pproj[idx % 2][:]) \
                              .then_inc(s_proj, 1)
                    if loop:
                        scalar.br(block.end_bb)

        @block.tensor
        def _(tensor):
            mainbb = nc.cur_bb
            from contextlib import nullcontext
            if loop:
                tensor.br("pe_p1")
            with tensor.register("pe_cnt") as pe_cnt, \
                 tensor.register("jp") as jp:
                with (nc.bb("pe_p1", parent=mainbb) if loop else nullcontext()):
                    for g in range(ng):
                        for m in range(MT):
                            idx = g * MT + m
                            if m == 0:
                                tensor.wait_ge(s_conv, n_loads + g + 1)
                            if idx >= 2:
                                tensor.wait_ge(s_pre, idx - 1)
                            for k in range(K):
                                mm = tensor.matmul(
                                    ppre[idx % 2][:],
                                    wxs_tile(k, m),
                                    xbf_r[:, k, :],
                                    start=(k == 0), stop=(k == K - 1),
                                )
                            mm.then_inc(s_zpre, 1)
                    tensor.wait_ge(s_pre, ng * MT)
                    tensor.reg_mov(pe_cnt, 1)
                    tensor.reg_mov(jp, 0)
                    if loop:
                        tensor.br("pe_chk")
                from contextlib import nullcontext
                with (nc.bb("pe_chk", parent=mainbb) if loop else nullcontext()):
                    if loop:
                        tensor.br_lt(jp, nit, "pe_body", "pe_p3")
                with (nc.bb("pe_body", parent=mainbb) if loop else nullcontext()):
                    for i in range(STEPS_PER_ITER if loop else t_total):
                        tensor.wait_ge(s_h, (i + 1) if (not loop or FORCE_CONST_WAITS) else pe_cnt)
                        for m in range(MT):
                            for k in range(K):
                                mm = tensor.matmul(
                                    zps[:, m:m + 1],
                                    whs_tile(k, m),
                                    hbf[:, k:k + 1],
                                    start=(k == 0), stop=(k == K - 1),
                                )
                        mm.then_inc(s_z, 1)
                        if loop:
                            tensor.reg_add(pe_cnt, pe_cnt, 1)
                    if loop:
                        tensor.reg_add(jp, jp, 1)
                        tensor.br("pe_chk")
                with (nc.bb("pe_p3", parent=mainbb) if loop else nullcontext()):
                    tensor.wait_ge(s_h, t_total + 1)
                    tensor.wait_ge(s_hist, t_total)
                    for m in range(PT):
                        for g in range(ng):
                            idx = m * ng + g
                            if idx >= 2:
                                tensor.wait_ge(s_proj, idx - 1)
                            for j in range(K):
                                mm = tensor.matmul(
                                    pproj[idx % 2][:],
                                    wps_tile(j, m),
                                    hist_r[:, j, g * 512:(g + 1) * 512],
                                    start=(j == 0), stop=(j == K - 1),
                                )
                            mm.then_inc(s_zproj, 1)
                    if loop:
                        tensor.br(block.end_bb)

    nc.compile()
    return nc


def make_in_maps(x, W_f, W_g, W_proj):
    Cv = C
    # wh is halved: the recurrence broadcasts hbf = 2*h, so (0.5*W_h)^T (2h) = W_h^T h
    wh_np = 0.5 * np.concatenate([W_f[Cv:], 2.0 * W_g[Cv:]], axis=1)
    wx_np = np.concatenate([W_f[:Cv], 2.0 * W_g[:Cv]], axis=1)
    wh_np = np.ascontiguousarray(wh_np, dtype=np.float32)
    wx_np = np.ascontiguousarray(wx_np, dtype=np.float32)
    wp_np = np.ascontiguousarray(W_proj, dtype=np.float32)
    in_maps = []
    for s in range(B):
        in_maps.append({
            "xt": np.ascontiguousarray(x[s].T, dtype=np.float32),
            "wh": wh_np,
            "wx": wx_np,
            "wp": wp_np,
        })
    return in_maps


_NC_CACHE = {}


def kernel(x, W_f, W_g, W_proj):
    key = x.shape
    if key not in _NC_CACHE:
        _NC_CACHE[key] = build_nc(x.shape[1], loop=False)
    nc = _NC_CACHE[key]
    in_maps = make_in_maps(np.asarray(x, dtype=np.float32),
                           np.asarray(W_f, dtype=np.float32),
                           np.asarray(W_g, dtype=np.float32),
                           np.asarray(W_proj, dtype=np.float32))
    res = bass_utils.run_bass_kernel_spmd(nc, in_maps, core_ids=list(range(B)))
    out = np.empty((B, x.shape[1], C), dtype=np.float32)
    for s in range(B):
        out[s] = res.results[s]["y"].T
    return out



# revision 8
# speedup vs baseline: 25.0203x; 25.0203x over previous
"""CfC recurrence kernel for Trainium2, 8 NeuronCores.

Sharding: data-parallel over batch B=8 (one sample per core); W_f/W_g/W_proj
replicated (shipped as 1/8 shards, all-gathered on device). The sequential
T=2048 recurrence is fully unrolled.

Host <-> device traffic is the bottleneck in this environment (axon relay,
~65 MB/s serialized, ~50ms per RPC), so the wrapper is built around:
  * bf16 everywhere on the wire (x 32MB in, y 32MB out, weights 10MB in)
  * one batched device_put (x raw + packed weight shards)
  * one small "prep" jit: per-core transpose of x, all-gather of weights
  * one cached custom-call jit for the Bass kernel (no per-call retrace /
    re-serialize / recompile)
  * y written t-major by the kernel so the host does no transpose

Per-core algorithm (sample s):
  phase 0: DMA bf16 weights into SBUF (3 DMAs)
  phase 1: precompute A[t, :] = x_t @ [W_fx | 2*W_gx]   (parallel over t)
  phase 2: sequential scan: z_t = A[t] + 0.5*[W_fh | 2*W_gh]^T (2 h_{t-1})
           u = tanh(0.5 z) ;  f = 0.5 u_f + 0.5, g = u_g
           h_t = 0.5 [ u_f (h-g) + h + g ]
           (the broadcast state is 2h in bf16 -- the p+q add writes it
            directly, W_fh/W_gh are pre-halved on the host to compensate)
  phase 3: y^T tiles via swapped operands: stationary = H tile, moving =
           W_proj  ->  psum [128 t, 1024 c]  ->  bf16  ->  contiguous DMA.

The sigmoid is computed via sigmoid(z) = 0.5 tanh(z/2) + 0.5 and the g-gate
weights are pre-doubled on the host so one Tanh activation (scale=0.5) covers
both gates.

Layouts (per core):
  xt     DRAM [1024, 2048] bf16  = x[s].T          (made by the prep jit)
  wh     DRAM [1024, 2048] bf16  = 0.5*[W_f[C:2C, :] | 2*W_g[C:2C, :]]
  wx     DRAM [1024, 2048] bf16  = [W_f[0:C, :]  | 2*W_g[0:C, :]]
  wproj  DRAM [1024, 1024] bf16
  y      DRAM [2048, 1024] bf16  = output [t, c_out] (t-major; no transpose)
"""

import sys

for _p in ("/opt/trn_rl_repo", "/root/.axon_site/_ro/trn_rl_repo"):
    if _p not in sys.path:
        sys.path.insert(0, _p)

import numpy as np
import ml_dtypes

from concourse import bass, bacc
import concourse.mybir as mybir

B, T, C = 8, 2048, 1024
K = 8          # c_in chunks of 128
MT = 16        # gate output tiles of 128 (8 f + 8 g)
TB = T // 128  # phase-3 time blocks
NG = T // 512  # 512-col groups for the phase-1 matmuls
F32 = mybir.dt.float32
BF16 = mybir.dt.bfloat16

BF = ml_dtypes.bfloat16
WPACK_ROWS = C + C + C // 2  # wh + wx + wp-as-[512,2048]


def build_nc(t_total=T):
    ng = t_total // 512
    tb_total = t_total // 128

    nc = bacc.Bacc("TRN2", target_bir_lowering=False, debug=False)

    xt = nc.dram_tensor("xt", [C, t_total], BF16, kind="ExternalInput")
    wh = nc.dram_tensor("wh", [C, 2 * C], BF16, kind="ExternalInput")
    wx = nc.dram_tensor("wx", [C, 2 * C], BF16, kind="ExternalInput")
    wp = nc.dram_tensor("wp", [C, C], BF16, kind="ExternalInput")
    y = nc.dram_tensor("y", [t_total, C], BF16, kind="ExternalOutput")

    # SBUF (bytes/partition):
    whs = nc.alloc_sbuf_tensor("whs", [128, K * 2 * C], BF16)       # 32KB/p
    wxs = nc.alloc_sbuf_tensor("wxs", [128, K * 2 * C], BF16)       # 32KB/p (reused as hist in phase 2+)
    wps = nc.alloc_sbuf_tensor("wps", [128, K * C], BF16)           # 16KB/p
    a_sb = nc.alloc_sbuf_tensor("a_sb", [128, t_total * MT], BF16)  # 64KB/p
    xbf = nc.alloc_sbuf_tensor("xbf", [128, 2 * K * 512], BF16)     # 16KB/p (2 group slots)
    h32 = nc.alloc_sbuf_tensor("h32", [128, 8], F32)
    hbf = nc.alloc_sbuf_tensor("hbf", [128, 8], BF16)
    za_sb = nc.alloc_sbuf_tensor("za_sb", [128, 32], F32)  # 2 slots of 16
    u_sb = nc.alloc_sbuf_tensor("u_sb", [128, 16], F32)
    d_sb = nc.alloc_sbuf_tensor("d_sb", [128, 8], F32)
    q_sb = nc.alloc_sbuf_tensor("q_sb", [128, 8], F32)
    p_sb = nc.alloc_sbuf_tensor("p_sb", [128, 8], F32)
    r_sb = nc.alloc_sbuf_tensor("r_sb", [128, 8], F32)
    ysb0 = nc.alloc_sbuf_tensor("ysb0", [128, C], BF16)
    ysb1 = nc.alloc_sbuf_tensor("ysb1", [128, C], BF16)
    ysb = [ysb0, ysb1]

    zps = nc.alloc_psum_tensor("zps", [128, 16], F32)
    ppre0 = nc.alloc_psum_tensor("ppre0", [128, 512], F32)
    ppre1 = nc.alloc_psum_tensor("ppre1", [128, 512], F32)
    ppre = [ppre0, ppre1]
    # 4 banks: [buffer][half] — a matmul output must stay inside one 2KB bank
    pproj = [[nc.alloc_psum_tensor(f"pproj{b}{h}", [128, 512], F32)
              for h in range(2)] for b in range(2)]

    s_w = nc.alloc_semaphore("s_w")       # weight DMAs (inc 16 each, 3 total)
    s_dx = nc.alloc_semaphore("s_dx")     # x group DMAs
    s_zpre = nc.alloc_semaphore("s_zpre")
    s_pre = nc.alloc_semaphore("s_pre")
    s_z = nc.alloc_semaphore("s_z")
    s_za = nc.alloc_semaphore("s_za")
    s_u = nc.alloc_semaphore("s_u")
    s_h = nc.alloc_semaphore("s_h")
    s_c1 = nc.alloc_semaphore("s_c1")
    s_c2 = nc.alloc_semaphore("s_c2")
    s_c3 = nc.alloc_semaphore("s_c3")
    s_c4 = nc.alloc_semaphore("s_c4")
    s_hist = nc.alloc_semaphore("s_hist")
    s_zproj = nc.alloc_semaphore("s_zproj")
    s_proj = nc.alloc_semaphore("s_proj")
    s_out0 = nc.alloc_semaphore("s_out0")
    s_out1 = nc.alloc_semaphore("s_out1")
    s_out = [s_out0, s_out1]

    def whs_tile(k, m):
        off = (k * MT + m) * 128
        return whs[:, off:off + 128]

    def wxs_tile(k, m):
        off = (k * MT + m) * 128
        return wxs[:, off:off + 128]

    # hist aliases wxs: [128, chunk(8), t] bf16 (chunk-major)
    hist_r = wxs.ap().rearrange("p (c t) -> p c t", c=K)
    a_r = a_sb.ap().rearrange("p (t m) -> p t m", m=MT)
    xbf_r = xbf.ap().rearrange("p (s c t) -> p s c t", s=2, c=K)
    hbf_3 = hbf.ap().rearrange("p (c o) -> p c o", o=1)

    with nc.Block() as block:

        @block.sync
        def _(sync):
            # phase 0: weights straight into SBUF (single DMA each)
            sync.dma_start(
                whs.ap().rearrange("p (k n) -> p k n", k=K),
                wh.ap().rearrange("(k p) n -> p k n", p=128),
            ).then_inc(s_w, 16)
            sync.dma_start(
                wxs.ap().rearrange("p (k n) -> p k n", k=K),
                wx.ap().rearrange("(k p) n -> p k n", p=128),
            ).then_inc(s_w, 16)
            sync.dma_start(
                wps.ap().rearrange("p (k n) -> p k n", k=K),
                wp.ap().rearrange("(k p) n -> p k n", p=128),
            ).then_inc(s_w, 16)
            # phase 1: x group loads (double-buffered slots)
            for g in range(ng):
                if g >= 2:
                    sync.wait_ge(s_zpre, MT * (g - 1))
                sync.dma_start(
                    xbf_r[:, g % 2, :, :],
                    xt[:, g * 512:(g + 1) * 512].rearrange(
                        "(c p) t -> p c t", p=128),
                ).then_inc(s_dx, 16)
            # phase 3: y tile stores
            for idx in range(tb_total):
                sync.wait_ge(s_proj, idx + 1)
                sync.dma_start(
                    y[idx * 128:(idx + 1) * 128, :],
                    ysb[idx % 2][:],
                ).then_inc(s_out[idx % 2], 16)
            sync.wait_ge(s_out[0], 16 * ((tb_total + 1) // 2))
            sync.wait_ge(s_out[1], 16 * (tb_total // 2))

        @block.vector
        def _(vector):
            # phase 2 preamble
            vector.memset(h32[:], 0.0)
            vector.memset(hbf[:], 0.0).then_inc(s_h, 1)
            for i in range(t_total):
                cnt = i + 1
                za_slot = za_sb[:, (i % 2) * 16:(i % 2) * 16 + 16]
                vector.wait_ge(s_z, cnt)
                vector.tensor_add(
                    za_slot, zps[:], a_sb[:, i * 16:(i + 1) * 16],
                ).then_inc(s_za, 1)
                vector.wait_ge(s_u, cnt)
                vector.wait_ge(s_hist, i)  # hist(t-1) done reading r_sb
                uf, ug = u_sb[:, 0:8], u_sb[:, 8:16]
                vector.tensor_sub(d_sb[:], h32[:], ug).then_inc(s_c1, 1)
                vector.tensor_add(q_sb[:], h32[:], ug).then_inc(s_c2, 1)
                vector.wait_ge(s_c1, cnt)
                vector.tensor_mul(p_sb[:], uf, d_sb[:]).then_inc(s_c3, 1)
                vector.wait_ge(s_c2, cnt)
                vector.wait_ge(s_c3, cnt)
                # hbf = p + q = 2*h in bf16; W_h is pre-halved on the host
                # so downstream matmuls see h exactly.
                vector.tensor_add(hbf[:], p_sb[:], q_sb[:]).then_inc(s_h, 1)
                vector.tensor_add(r_sb[:], p_sb[:], q_sb[:]).then_inc(s_c4, 1)
                vector.wait_ge(s_c4, cnt)
                vector.tensor_scalar_mul(h32[:], r_sb[:], 0.5)
                r_3 = r_sb.ap().rearrange("p (c o) -> p c o", o=1)
                vector.wait_ge(s_c4, cnt)
                vector.tensor_scalar_mul(
                    hist_r[:, :, i:i + 1], r_3, 0.5).then_inc(s_hist, 1)

        @block.scalar
        def _(scalar):
            # phase 1: psum -> a_sb copies
            for idx in range(ng * MT):
                g, m = idx // MT, idx % MT
                scalar.wait_ge(s_zpre, idx + 1)
                scalar.copy(
                    a_r[:, g * 512:(g + 1) * 512, m],
                    ppre[idx % 2][:],
                ).then_inc(s_pre, 1)
            # phase 2: the tanh
            for i in range(t_total):
                scalar.wait_ge(s_za, i + 1)
                zbase = (i % 2) * 16
                scalar.activation(
                    u_sb[:], za_sb[:, zbase:zbase + 16],
                    mybir.ActivationFunctionType.Tanh, scale=0.5,
                ).then_inc(s_u, 1)
            # phase 3: psum -> bf16 staging for the y stores
            for idx in range(tb_total):
                scalar.wait_ge(s_zproj, idx + 1)
                if idx >= 2:
                    scalar.wait_ge(s_out[idx % 2], 16 * ((idx - 2) // 2 + 1))
                scalar.copy(ysb[idx % 2][:, 0:512], pproj[idx % 2][0][:])
                scalar.copy(ysb[idx % 2][:, 512:1024], pproj[idx % 2][1][:]) \
                      .then_inc(s_proj, 1)

        @block.tensor
        def _(tensor):
            # phase 1: A = W_x^T x, 512 t at a time
            for g in range(ng):
                for m in range(MT):
                    idx = g * MT + m
                    if m == 0:
                        tensor.wait_ge(s_dx, 16 * (g + 1))
                        if g == 0:
                            tensor.wait_ge(s_w, 48)  # all weight DMAs done
                    if idx >= 2:
                        tensor.wait_ge(s_pre, idx - 1)
                    for k in range(K):
                        mm = tensor.matmul(
                            ppre[idx % 2][:],
                            wxs_tile(k, m),
                            xbf_r[:, g % 2, k, :],
                            start=(k == 0), stop=(k == K - 1),
                        )
                    mm.then_inc(s_zpre, 1)
            tensor.wait_ge(s_pre, ng * MT)
            # phase 2: z_h = (0.5 W_h)^T (2 h)
            for i in range(t_total):
                tensor.wait_ge(s_h, i + 1)
                for m in range(MT):
                    for k in range(K):
                        mm = tensor.matmul(
                            zps[:, m:m + 1],
                            whs_tile(k, m),
                            hbf[:, k:k + 1],
                            start=(k == 0), stop=(k == K - 1),
                        )
                mm.then_inc(s_z, 1)
            # phase 3: y^T blocks: stationary = hist tile, moving = W_proj
            tensor.wait_ge(s_h, t_total + 1)
            tensor.wait_ge(s_hist, t_total)
            tensor.wait_ge(s_w, 48)  # wps loaded
            for idx in range(tb_total):
                if idx >= 2:
                    tensor.wait_ge(s_proj, idx - 1)
                for h in range(2):
                    for j in range(K):
                        mm = tensor.matmul(
                            pproj[idx % 2][h][:],
                            hist_r[:, j, idx * 128:(idx + 1) * 128],
                            wps[:, j * C + h * 512:j * C + h * 512 + 512],
                            start=(j == 0), stop=(j == K - 1),
                        )
                mm.then_inc(s_zproj, 1)

    nc.compile()
    return nc


# ---------------------------------------------------------------- jax glue

_STATE = {}


def _get_state(t_total):
    if t_total in _STATE:
        return _STATE[t_total]
    import jax
    import jax.numpy as jnp
    from jax.sharding import Mesh, PartitionSpec as P, NamedSharding
    from jax.experimental.shard_map import shard_map
    from concourse import bass2jax
    from concourse.bass2jax import _bass_exec_p, partition_id_tensor

    bass2jax.install_neuronx_cc_hook()

    nc = build_nc(t_total)

    devs = jax.devices()[:B]
    mesh = Mesh(np.asarray(devs), ("core",))
    shard = NamedSharding(mesh, P("core"))

    # names in allocation order (must match the declarations in build_nc);
    # operands are inputs + output buffer + partition id, as in
    # bass2jax.run_bass_via_pjrt
    part_name = nc.partition_id_tensor.name if nc.partition_id_tensor else None
    in_names = ("xt", "wh", "wx", "wp", "y") + ((part_name,) if part_name else ())
    out_names = ("y",)
    out_avals = (jax.core.ShapedArray((t_total, C), jnp.bfloat16),)

    def _prep(x_loc, w_loc):
        # x_loc [t_total, 1024] bf16; w_loc [WPACK_ROWS//8, 2048] bf16
        xt = x_loc.T
        w = jax.lax.all_gather(w_loc, "core", axis=0, tiled=True)
        whv = w[0:C]
        wxv = w[C:2 * C]
        wpv = w[2 * C:].reshape(C, C)
        return xt, whv, wxv, wpv

    prep = jax.jit(shard_map(
        _prep, mesh=mesh,
        in_specs=(P("core"), P("core")),
        out_specs=(P("core"), P("core"), P("core"), P("core")),
        check_rep=False,
    ))

    def _body(*args):
        operands = list(args)
        operands.append(partition_id_tensor())
        outs = _bass_exec_p.bind(
            *operands,
            out_avals=out_avals,
            in_names=in_names,
            out_names=out_names,
            lowering_input_output_aliases=(),
            sim_require_finite=True,
            sim_require_nnan=True,
            nc=nc,
        )
        return tuple(outs)

    kern = jax.jit(shard_map(
        _body, mesh=mesh,
        in_specs=(P("core"),) * 5,
        out_specs=(P("core"),),
        check_rep=False,
    ), keep_unused=True)

    # persistent dummy output operand (the kernel writes every element of y,
    # so its initial contents never matter; not donated so it is reusable)
    zjit = jax.jit(lambda: jnp.zeros((B * t_total, C), jnp.bfloat16),
                   out_shardings=shard)
    ydummy = jax.block_until_ready(zjit())

    st = {"nc": nc, "mesh": mesh, "shard": shard, "prep": prep,
          "kern": kern, "ydummy": ydummy, "jax": jax}
    _STATE[t_total] = st
    return st


def _pack_weights(W_f, W_g, W_proj):
    Cv = C
    wh_np = 0.5 * np.concatenate([W_f[Cv:], 2.0 * W_g[Cv:]], axis=1)
    wx_np = np.concatenate([W_f[:Cv], 2.0 * W_g[:Cv]], axis=1)
    pack = np.empty((WPACK_ROWS, 2 * Cv), dtype=BF)
    pack[0:Cv] = wh_np.astype(BF)
    pack[Cv:2 * Cv] = wx_np.astype(BF)
    pack[2 * Cv:] = W_proj.astype(BF).reshape(Cv // 2, 2 * Cv)
    return pack


def kernel(x, W_f, W_g, W_proj):
    t_total = x.shape[1]
    st = _get_state(t_total)
    jax = st["jax"]

    xg = np.ascontiguousarray(x, dtype=np.float32).astype(BF) \
           .reshape(B * t_total, C)
    wpack = _pack_weights(np.asarray(W_f, np.float32),
                          np.asarray(W_g, np.float32),
                          np.asarray(W_proj, np.float32))

    xd, wd = jax.device_put([xg, wpack], [st["shard"], st["shard"]])
    xt, whv, wxv, wpv = st["prep"](xd, wd)
    (y,) = st["kern"](xt, whv, wxv, wpv, st["ydummy"])
    yh = np.asarray(y)
    return yh.reshape(B, t_total, C).astype(np.float32)


# revision 29
# speedup vs baseline: 32.1997x; 1.2869x over previous
"""CfC recurrence kernel for Trainium2, 8 NeuronCores.

Sharding: data-parallel over batch B=8 (one sample per core); W_f/W_g/W_proj
replicated (shipped as 1/8 shards, all-gathered on device). The sequential
T=2048 recurrence is fully unrolled.

Host <-> device traffic is the bottleneck in this environment (axon relay,
~65 MB/s serialized, ~50ms per RPC), so the wrapper is built around:
  * bf16 everywhere on the wire (x 32MB in, y 32MB out, weights 10MB in)
  * one batched device_put (x raw + packed weight shards)
  * one small "prep" jit: per-core transpose of x, all-gather of weights
  * one cached custom-call jit for the Bass kernel (no per-call retrace /
    re-serialize / recompile)
  * y written t-major by the kernel so the host does no transpose

Per-core algorithm (sample s):
  phase 0: DMA bf16 weights into SBUF (3 DMAs)
  phase 1: precompute A[t, :] = x_t @ [W_fx | 2*W_gx]   (parallel over t)
  phase 2: sequential scan: z_t = A[t] + 0.5*[W_fh | 2*W_gh]^T (2 h_{t-1})
           u = tanh(0.5 z) ;  f = 0.5 u_f + 0.5, g = u_g
           h_t = 0.5 [ u_f (h-g) + h + g ]
           (the broadcast state is 2h in bf16 -- the p+q add writes it
            directly, W_fh/W_gh are pre-halved on the host to compensate)
  phase 3: y^T tiles via swapped operands: stationary = H tile, moving =
           W_proj  ->  psum [128 t, 1024 c]  ->  bf16  ->  contiguous DMA.

The sigmoid is computed via sigmoid(z) = 0.5 tanh(z/2) + 0.5 and the g-gate
weights are pre-doubled on the host so one Tanh activation (scale=0.5) covers
both gates.

Layouts (per core):
  xt     DRAM [1024, 2048] bf16  = x[s].T          (made by the prep jit)
  wh     DRAM [1024, 2048] bf16  = 0.5*[W_f[C:2C, :] | 2*W_g[C:2C, :]]
  wx     DRAM [1024, 2048] bf16  = [W_f[0:C, :]  | 2*W_g[0:C, :]]
  wproj  DRAM [1024, 1024] bf16
  y      DRAM [2048, 1024] bf16  = output [t, c_out] (t-major; no transpose)
"""

import sys

for _p in ("/opt/trn_rl_repo", "/root/.axon_site/_ro/trn_rl_repo"):
    if _p not in sys.path:
        sys.path.insert(0, _p)

import numpy as np
import ml_dtypes

from concourse import bass, bacc
import concourse.mybir as mybir

B, T, C = 8, 2048, 1024
K = 8          # c_in chunks of 128
MT = 16        # gate output tiles of 128 (8 f + 8 g)
TB = T // 128  # phase-3 time blocks
NG = T // 512  # 512-col groups for the phase-1 matmuls
F32 = mybir.dt.float32
BF16 = mybir.dt.bfloat16

BF = ml_dtypes.bfloat16
WPACK_ROWS = C + C + C // 2  # wh + wx + wp-as-[512,2048]

# Ship x as per-channel-scaled int8 (scales folded into the x-side weights on
# the host): halves the x wire bytes, but on the real jax key(0) inputs the
# recurrence amplifies the quantization noise to rel err 0.024 > 2e-2 gate
# (rng-seeded sim said 0.011) — so this stays OFF.
X_INT8 = False
X_DT = mybir.dt.int8 if X_INT8 else BF16
X_NP = np.int8 if X_INT8 else BF


def build_nc(t_total=T):
    ng = t_total // 512
    tb_total = t_total // 128

    nc = bacc.Bacc("TRN2", target_bir_lowering=False, debug=False)

    xt = nc.dram_tensor("xt", [C, t_total], X_DT, kind="ExternalInput")
    wh = nc.dram_tensor("wh", [C, 2 * C], BF16, kind="ExternalInput")
    wx = nc.dram_tensor("wx", [C, 2 * C], BF16, kind="ExternalInput")
    wp = nc.dram_tensor("wp", [C, C], BF16, kind="ExternalInput")
    y = nc.dram_tensor("y", [t_total, C], BF16, kind="ExternalOutput")

    # SBUF (bytes/partition):
    whs = nc.alloc_sbuf_tensor("whs", [128, K * 2 * C], BF16)       # 32KB/p
    wxs = nc.alloc_sbuf_tensor("wxs", [128, K * 2 * C], BF16)       # 32KB/p (reused as hist in phase 2+)
    wps = nc.alloc_sbuf_tensor("wps", [128, K * C], BF16)           # 16KB/p
    a_sb = nc.alloc_sbuf_tensor("a_sb", [128, t_total * MT], BF16)  # 64KB/p
    xbf = nc.alloc_sbuf_tensor("xbf", [128, 2 * K * 512], BF16)     # 16KB/p (2 group slots)
    if X_INT8:
        xi8 = nc.alloc_sbuf_tensor("xi8", [128, 2 * K * 512], X_DT)  # 8KB/p
    h32 = nc.alloc_sbuf_tensor("h32", [128, 8], F32)
    hbf = nc.alloc_sbuf_tensor("hbf", [128, 8], BF16)
    za_sb = nc.alloc_sbuf_tensor("za_sb", [128, 32], F32)  # 2 slots of 16
    u_sb = nc.alloc_sbuf_tensor("u_sb", [128, 16], F32)
    d_sb = nc.alloc_sbuf_tensor("d_sb", [128, 8], F32)
    q_sb = nc.alloc_sbuf_tensor("q_sb", [128, 8], F32)
    p_sb = nc.alloc_sbuf_tensor("p_sb", [128, 8], F32)
    r_sb = nc.alloc_sbuf_tensor("r_sb", [128, 8], F32)
    ysb0 = nc.alloc_sbuf_tensor("ysb0", [128, C], BF16)
    ysb1 = nc.alloc_sbuf_tensor("ysb1", [128, C], BF16)
    ysb = [ysb0, ysb1]

    zps = nc.alloc_psum_tensor("zps", [128, 16], F32)
    ppre0 = nc.alloc_psum_tensor("ppre0", [128, 512], F32)
    ppre1 = nc.alloc_psum_tensor("ppre1", [128, 512], F32)
    ppre = [ppre0, ppre1]
    # 4 banks: [buffer][half] — a matmul output must stay inside one 2KB bank
    pproj = [[nc.alloc_psum_tensor(f"pproj{b}{h}", [128, 512], F32)
              for h in range(2)] for b in range(2)]

    s_w = nc.alloc_semaphore("s_w")       # weight DMAs (inc 16 each, 3 total)
    s_dx = nc.alloc_semaphore("s_dx")     # x group DMAs
    if X_INT8:
        s_cvx = nc.alloc_semaphore("s_cvx")  # x int8 -> bf16 group converts
    s_zpre = nc.alloc_semaphore("s_zpre")
    s_pre = nc.alloc_semaphore("s_pre")
    s_z = nc.alloc_semaphore("s_z")
    s_za = nc.alloc_semaphore("s_za")
    s_u = nc.alloc_semaphore("s_u")
    s_h = nc.alloc_semaphore("s_h")
    s_c1 = nc.alloc_semaphore("s_c1")
    s_c2 = nc.alloc_semaphore("s_c2")
    s_c3 = nc.alloc_semaphore("s_c3")
    s_c4 = nc.alloc_semaphore("s_c4")
    s_hist = nc.alloc_semaphore("s_hist")
    s_zproj = nc.alloc_semaphore("s_zproj")
    s_proj = nc.alloc_semaphore("s_proj")
    s_out0 = nc.alloc_semaphore("s_out0")
    s_out1 = nc.alloc_semaphore("s_out1")
    s_out = [s_out0, s_out1]

    def whs_tile(k, m):
        off = (k * MT + m) * 128
        return whs[:, off:off + 128]

    def wxs_tile(k, m):
        off = (k * MT + m) * 128
        return wxs[:, off:off + 128]

    # hist aliases wxs: [128, chunk(8), t] bf16 (chunk-major)
    hist_r = wxs.ap().rearrange("p (c t) -> p c t", c=K)
    a_r = a_sb.ap().rearrange("p (t m) -> p t m", m=MT)
    xbf_r = xbf.ap().rearrange("p (s c t) -> p s c t", s=2, c=K)
    xdma = xi8 if X_INT8 else xbf
    xdma_r = xdma.ap().rearrange("p (s c t) -> p s c t", s=2, c=K)
    hbf_3 = hbf.ap().rearrange("p (c o) -> p c o", o=1)

    with nc.Block() as block:

        @block.sync
        def _(sync):
            # phase 0: weights straight into SBUF (single DMA each)
            sync.dma_start(
                whs.ap().rearrange("p (k n) -> p k n", k=K),
                wh.ap().rearrange("(k p) n -> p k n", p=128),
            ).then_inc(s_w, 16)
            sync.dma_start(
                wxs.ap().rearrange("p (k n) -> p k n", k=K),
                wx.ap().rearrange("(k p) n -> p k n", p=128),
            ).then_inc(s_w, 16)
            sync.dma_start(
                wps.ap().rearrange("p (k n) -> p k n", k=K),
                wp.ap().rearrange("(k p) n -> p k n", p=128),
            ).then_inc(s_w, 16)
            # phase 1: x group loads (double-buffered slots)
            for g in range(ng):
                if g >= 2:
                    if X_INT8:
                        sync.wait_ge(s_cvx, g - 1)
                    else:
                        sync.wait_ge(s_zpre, MT * (g - 1))
                sync.dma_start(
                    xdma_r[:, g % 2, :, :],
                    xt[:, g * 512:(g + 1) * 512].rearrange(
                        "(c p) t -> p c t", p=128),
                ).then_inc(s_dx, 16)
            # phase 3: y tile stores
            for idx in range(tb_total):
                sync.wait_ge(s_proj, idx + 1)
                sync.dma_start(
                    y[idx * 128:(idx + 1) * 128, :],
                    ysb[idx % 2][:],
                ).then_inc(s_out[idx % 2], 16)
            sync.wait_ge(s_out[0], 16 * ((tb_total + 1) // 2))
            sync.wait_ge(s_out[1], 16 * (tb_total // 2))

        @block.vector
        def _(vector):
            # phase 1: x int8 -> bf16 group converts (vector is idle here)
            if X_INT8:
                for g in range(ng):
                    vector.wait_ge(s_dx, 16 * (g + 1))
                    if g >= 2:
                        vector.wait_ge(s_zpre, MT * (g - 1))
                    gb = (g % 2) * K * 512
                    vector.tensor_copy(
                        xbf[:, gb:gb + K * 512], xi8[:, gb:gb + K * 512],
                    ).then_inc(s_cvx, 1)
            # phase 2 preamble
            vector.memset(h32[:], 0.0)
            vector.memset(hbf[:], 0.0).then_inc(s_h, 1)
            for i in range(t_total):
                cnt = i + 1
                za_slot = za_sb[:, (i % 2) * 16:(i % 2) * 16 + 16]
                vector.wait_ge(s_z, cnt)
                vector.tensor_add(
                    za_slot, zps[:], a_sb[:, i * 16:(i + 1) * 16],
                ).then_inc(s_za, 1)
                vector.wait_ge(s_u, cnt)
                vector.wait_ge(s_hist, i)  # hist(t-1) done reading r_sb
                uf, ug = u_sb[:, 0:8], u_sb[:, 8:16]
                vector.tensor_sub(d_sb[:], h32[:], ug).then_inc(s_c1, 1)
                vector.tensor_add(q_sb[:], h32[:], ug).then_inc(s_c2, 1)
                vector.wait_ge(s_c1, cnt)
                vector.tensor_mul(p_sb[:], uf, d_sb[:]).then_inc(s_c3, 1)
                vector.wait_ge(s_c2, cnt)
                vector.wait_ge(s_c3, cnt)
                # hbf = p + q = 2*h in bf16; W_h is pre-halved on the host
                # so downstream matmuls see h exactly.
                vector.tensor_add(hbf[:], p_sb[:], q_sb[:]).then_inc(s_h, 1)
                vector.tensor_add(r_sb[:], p_sb[:], q_sb[:]).then_inc(s_c4, 1)
                vector.wait_ge(s_c4, cnt)
                vector.tensor_scalar_mul(h32[:], r_sb[:], 0.5)
                r_3 = r_sb.ap().rearrange("p (c o) -> p c o", o=1)
                vector.wait_ge(s_c4, cnt)
                vector.tensor_scalar_mul(
                    hist_r[:, :, i:i + 1], r_3, 0.5).then_inc(s_hist, 1)

        @block.scalar
        def _(scalar):
            # phase 1: psum -> a_sb copies
            for idx in range(ng * MT):
                g, m = idx // MT, idx % MT
                scalar.wait_ge(s_zpre, idx + 1)
                scalar.copy(
                    a_r[:, g * 512:(g + 1) * 512, m],
                    ppre[idx % 2][:],
                ).then_inc(s_pre, 1)
            # phase 2: the tanh
            for i in range(t_total):
                scalar.wait_ge(s_za, i + 1)
                zbase = (i % 2) * 16
                scalar.activation(
                    u_sb[:], za_sb[:, zbase:zbase + 16],
                    mybir.ActivationFunctionType.Tanh, scale=0.5,
                ).then_inc(s_u, 1)
            # phase 3: psum -> bf16 staging for the y stores
            for idx in range(tb_total):
                scalar.wait_ge(s_zproj, idx + 1)
                if idx >= 2:
                    scalar.wait_ge(s_out[idx % 2], 16 * ((idx - 2) // 2 + 1))
                scalar.copy(ysb[idx % 2][:, 0:512], pproj[idx % 2][0][:])
                scalar.copy(ysb[idx % 2][:, 512:1024], pproj[idx % 2][1][:]) \
                      .then_inc(s_proj, 1)

        @block.tensor
        def _(tensor):
            # phase 1: A = W_x^T x, 512 t at a time
            for g in range(ng):
                for m in range(MT):
                    idx = g * MT + m
                    if m == 0:
                        if X_INT8:
                            tensor.wait_ge(s_cvx, g + 1)
                        else:
                            tensor.wait_ge(s_dx, 16 * (g + 1))
                        if g == 0:
                            tensor.wait_ge(s_w, 48)  # all weight DMAs done
                    if idx >= 2:
                        tensor.wait_ge(s_pre, idx - 1)
                    for k in range(K):
                        mm = tensor.matmul(
                            ppre[idx % 2][:],
                            wxs_tile(k, m),
                            xbf_r[:, g % 2, k, :],
                            start=(k == 0), stop=(k == K - 1),
                        )
                    mm.then_inc(s_zpre, 1)
            tensor.wait_ge(s_pre, ng * MT)
            # phase 2: z_h = (0.5 W_h)^T (2 h)
            for i in range(t_total):
                tensor.wait_ge(s_h, i + 1)
                for m in range(MT):
                    for k in range(K):
                        mm = tensor.matmul(
                            zps[:, m:m + 1],
                            whs_tile(k, m),
                            hbf[:, k:k + 1],
                            start=(k == 0), stop=(k == K - 1),
                        )
                mm.then_inc(s_z, 1)
            # phase 3: y^T blocks: stationary = hist tile, moving = W_proj
            tensor.wait_ge(s_h, t_total + 1)
            tensor.wait_ge(s_hist, t_total)
            tensor.wait_ge(s_w, 48)  # wps loaded
            for idx in range(tb_total):
                if idx >= 2:
                    tensor.wait_ge(s_proj, idx - 1)
                for h in range(2):
                    for j in range(K):
                        mm = tensor.matmul(
                            pproj[idx % 2][h][:],
                            hist_r[:, j, idx * 128:(idx + 1) * 128],
                            wps[:, j * C + h * 512:j * C + h * 512 + 512],
                            start=(j == 0), stop=(j == K - 1),
                        )
                mm.then_inc(s_zproj, 1)

    nc.compile()
    return nc


# ---------------------------------------------------------------- jax glue

_STATE = {}


def _get_state(t_total):
    if t_total in _STATE:
        return _STATE[t_total]
    import jax
    import jax.numpy as jnp
    from jax.sharding import Mesh, PartitionSpec as P, NamedSharding
    from jax.experimental.shard_map import shard_map
    from concourse import bass2jax
    from concourse.bass2jax import _bass_exec_p, partition_id_tensor

    bass2jax.install_neuronx_cc_hook()

    nc = build_nc(t_total)

    devs = jax.devices()[:B]
    mesh = Mesh(np.asarray(devs), ("core",))
    shard = NamedSharding(mesh, P("core"))

    # names in allocation order (must match the declarations in build_nc);
    # operands are inputs + output buffer + partition id, as in
    # bass2jax.run_bass_via_pjrt
    part_name = nc.partition_id_tensor.name if nc.partition_id_tensor else None
    in_names = ("xt", "wh", "wx", "wp", "y") + ((part_name,) if part_name else ())
    out_names = ("y",)
    out_avals = (jax.core.ShapedArray((t_total, C), jnp.bfloat16),)

    def _prep_x(x1_loc, x2_loc):
        # x halves [t_total//2, 1024] bf16 -> xt [1024, t_total]
        return jnp.concatenate([x1_loc.T, x2_loc.T], axis=1)

    prep_x = jax.jit(shard_map(
        _prep_x, mesh=mesh,
        in_specs=(P("core"), P("core")),
        out_specs=P("core"),
        check_rep=False,
    ))

    def _prep_w(w_loc):
        # w_loc [WPACK_ROWS//8, 2048] bf16
        w = jax.lax.all_gather(w_loc, "core", axis=0, tiled=True)
        whv = w[0:C]
        wxv = w[C:2 * C]
        wpv = w[2 * C:].reshape(C, C)
        return whv, wxv, wpv

    prep_w = jax.jit(shard_map(
        _prep_w, mesh=mesh,
        in_specs=(P("core"),),
        out_specs=(P("core"), P("core"), P("core")),
        check_rep=False,
    ))

    def _body(*args):
        operands = list(args)
        operands.append(partition_id_tensor())
        outs = _bass_exec_p.bind(
            *operands,
            out_avals=out_avals,
            in_names=in_names,
            out_names=out_names,
            lowering_input_output_aliases=(),
            sim_require_finite=True,
            sim_require_nnan=True,
            nc=nc,
        )
        return tuple(outs)

    kern = jax.jit(shard_map(
        _body, mesh=mesh,
        in_specs=(P("core"),) * 5,
        out_specs=(P("core"),),
        check_rep=False,
    ), keep_unused=True)

    # persistent dummy output operand (the kernel writes every element of y,
    # so its initial contents never matter; not donated so it is reusable)
    zjit = jax.jit(lambda: jnp.zeros((B * t_total, C), jnp.bfloat16),
                   out_shardings=shard)
    ydummy = jax.block_until_ready(zjit())

    from concurrent.futures import ThreadPoolExecutor
    st = {"nc": nc, "mesh": mesh, "shard": shard,
          "prep_x": prep_x, "prep_w": prep_w,
          "kern": kern, "ydummy": ydummy, "jax": jax,
          "pool": ThreadPoolExecutor(8),
          "wkey": None, "wdev": None}
    _STATE[t_total] = st
    return st


def _pack_weights(W_f, W_g, W_proj, x_scale=None):
    Cv = C
    wh_np = 0.5 * np.concatenate([W_f[Cv:], 2.0 * W_g[Cv:]], axis=1)
    wx_np = np.concatenate([W_f[:Cv], 2.0 * W_g[:Cv]], axis=1)
    if x_scale is not None:
        wx_np *= x_scale[:, None]  # fold the int8 x dequant scales
    pack = np.empty((WPACK_ROWS, 2 * Cv), dtype=BF)
    pack[0:Cv] = wh_np.astype(BF)
    pack[Cv:2 * Cv] = wx_np.astype(BF)
    pack[2 * Cv:] = W_proj.astype(BF).reshape(Cv // 2, 2 * Cv)
    return pack


def kernel(x, W_f, W_g, W_proj):
    t_total = x.shape[1]
    st = _get_state(t_total)
    jax = st["jax"]
    sh = st["shard"]
    th = t_total // 2

    # Weights are usually identical across calls: keep their device-resident,
    # all-gathered form cached, keyed on a content hash (cheap vs 10.5MB of
    # relay traffic + packing).
    import hashlib
    hsh = hashlib.blake2b(digest_size=16)
    W_f = np.ascontiguousarray(W_f, np.float32)
    W_g = np.ascontiguousarray(W_g, np.float32)
    W_proj = np.ascontiguousarray(W_proj, np.float32)
    for w in (W_f, W_g, W_proj):
        hsh.update(w)
    wkey = hsh.digest()
    if st["wkey"] != wkey:
        wpack = _pack_weights(W_f, W_g, W_proj)
        wd = jax.device_put(wpack, sh)
        st["wdev"] = jax.block_until_ready(st["prep_w"](wd))
        st["wkey"] = wkey
    whv, wxv, wpv = st["wdev"]

    # One batched device_put: on this 1-core host, splitting transfers to
    # overlap with numpy work regresses (the relay's client pump and numpy
    # contend for the single CPU), so keep host prep and transfer sequential.
    x = np.ascontiguousarray(x, dtype=np.float32)
    x1 = x[:, :th].astype(BF).reshape(B * th, C)
    x2 = x[:, th:].astype(BF).reshape(B * th, C)
    x1d, x2d = jax.device_put([x1, x2], [sh, sh])

    xt = st["prep_x"](x1d, x2d)
    (y,) = st["kern"](xt, whv, wxv, wpv, st["ydummy"])

    # Fetch y shard-by-shard (the relay serializes anyway) and overlap the
    # bf16 -> f32 conversion of shard i with the fetch of shard i+1.
    out = np.empty((B, t_total, C), dtype=np.float32)
    shards = y.addressable_shards
    futs = [(s.index[0].start // t_total, st["pool"].submit(np.asarray, s.data))
            for s in shards]
    for core, fut in futs:
        out[core] = fut.result()
    return out


# revision 31
# speedup vs baseline: 36.6776x; 1.1391x over previous
"""CfC recurrence kernel for Trainium2, 8 NeuronCores.

Sharding: data-parallel over batch B=8 (one sample per core); W_f/W_g/W_proj
replicated (shipped as 1/8 shards, all-gathered on device). The sequential
T=2048 recurrence is fully unrolled.

Host <-> device traffic is the bottleneck in this environment (axon relay,
~65 MB/s serialized, ~50ms per RPC), so the wrapper is built around:
  * bf16 everywhere on the wire (x 32MB in, y 32MB out, weights 10MB in)
  * one batched device_put (x raw + packed weight shards)
  * one small "prep" jit: per-core transpose of x, all-gather of weights
  * one cached custom-call jit for the Bass kernel (no per-call retrace /
    re-serialize / recompile)
  * y written t-major by the kernel so the host does no transpose

Per-core algorithm (sample s):
  phase 0: DMA bf16 weights into SBUF (3 DMAs)
  phase 1: precompute A[t, :] = x_t @ [W_fx | 2*W_gx]   (parallel over t)
  phase 2: sequential scan: z_t = A[t] + 0.5*[W_fh | 2*W_gh]^T (2 h_{t-1})
           u = tanh(0.5 z) ;  f = 0.5 u_f + 0.5, g = u_g
           h_t = 0.5 [ u_f (h-g) + h + g ]
           (the broadcast state is 2h in bf16 -- the p+q add writes it
            directly, W_fh/W_gh are pre-halved on the host to compensate)
  phase 3: y^T tiles via swapped operands: stationary = H tile, moving =
           W_proj  ->  psum [128 t, 1024 c]  ->  bf16  ->  contiguous DMA.

The sigmoid is computed via sigmoid(z) = 0.5 tanh(z/2) + 0.5 and the g-gate
weights are pre-doubled on the host so one Tanh activation (scale=0.5) covers
both gates.

Layouts (per core):
  xt     DRAM [1024, 2048] bf16  = x[s].T          (made by the prep jit)
  wh     DRAM [1024, 2048] bf16  = 0.5*[W_f[C:2C, :] | 2*W_g[C:2C, :]]
  wx     DRAM [1024, 2048] bf16  = [W_f[0:C, :]  | 2*W_g[0:C, :]]
  wproj  DRAM [1024, 1024] bf16
  y      DRAM [2048, 1024] bf16  = output [t, c_out] (t-major; no transpose)
"""

import sys

for _p in ("/opt/trn_rl_repo", "/root/.axon_site/_ro/trn_rl_repo"):
    if _p not in sys.path:
        sys.path.insert(0, _p)

import numpy as np
import ml_dtypes

from concourse import bass, bacc
import concourse.mybir as mybir

B, T, C = 8, 2048, 1024
K = 8          # c_in chunks of 128
MT = 16        # gate output tiles of 128 (8 f + 8 g)
TB = T // 128  # phase-3 time blocks
NG = T // 512  # 512-col groups for the phase-1 matmuls
F32 = mybir.dt.float32
BF16 = mybir.dt.bfloat16

BF = ml_dtypes.bfloat16
WPACK_ROWS = C + C + C // 2  # wh + wx + wp-as-[512,2048]

# Ship x as per-channel-scaled int8 (scales folded into the x-side weights on
# the host): halves the x wire bytes, but on the real jax key(0) inputs the
# recurrence amplifies the quantization noise to rel err 0.024 > 2e-2 gate
# (rng-seeded sim said 0.011) — so this stays OFF.
X_INT8 = False
X_DT = mybir.dt.int8 if X_INT8 else BF16
X_NP = np.int8 if X_INT8 else BF


def build_nc(t_total=T):
    ng = t_total // 512
    tb_total = t_total // 128

    nc = bacc.Bacc("TRN2", target_bir_lowering=False, debug=False)

    xt = nc.dram_tensor("xt", [C, t_total], X_DT, kind="ExternalInput")
    wh = nc.dram_tensor("wh", [C, 2 * C], BF16, kind="ExternalInput")
    wx = nc.dram_tensor("wx", [C, 2 * C], BF16, kind="ExternalInput")
    wp = nc.dram_tensor("wp", [C, C], BF16, kind="ExternalInput")
    y = nc.dram_tensor("y", [t_total, C], BF16, kind="ExternalOutput")

    # SBUF (bytes/partition):
    whs = nc.alloc_sbuf_tensor("whs", [128, K * 2 * C], BF16)       # 32KB/p
    wxs = nc.alloc_sbuf_tensor("wxs", [128, K * 2 * C], BF16)       # 32KB/p (reused as hist in phase 2+)
    wps = nc.alloc_sbuf_tensor("wps", [128, K * C], BF16)           # 16KB/p
    a_sb = nc.alloc_sbuf_tensor("a_sb", [128, t_total * MT], BF16)  # 64KB/p
    xbf = nc.alloc_sbuf_tensor("xbf", [128, 2 * K * 512], BF16)     # 16KB/p (2 group slots)
    if X_INT8:
        xi8 = nc.alloc_sbuf_tensor("xi8", [128, 2 * K * 512], X_DT)  # 8KB/p
    h32 = nc.alloc_sbuf_tensor("h32", [128, 8], F32)
    hbf = nc.alloc_sbuf_tensor("hbf", [128, 8], BF16)
    za_sb = nc.alloc_sbuf_tensor("za_sb", [128, 32], F32)  # 2 slots of 16
    u_sb = nc.alloc_sbuf_tensor("u_sb", [128, 16], F32)
    d_sb = nc.alloc_sbuf_tensor("d_sb", [128, 8], F32)
    q_sb = nc.alloc_sbuf_tensor("q_sb", [128, 8], F32)
    p_sb = nc.alloc_sbuf_tensor("p_sb", [128, 8], F32)
    r_sb = nc.alloc_sbuf_tensor("r_sb", [128, 8], F32)
    ysb0 = nc.alloc_sbuf_tensor("ysb0", [128, C], BF16)
    ysb1 = nc.alloc_sbuf_tensor("ysb1", [128, C], BF16)
    ysb = [ysb0, ysb1]

    zps = nc.alloc_psum_tensor("zps", [128, 16], F32)
    ppre0 = nc.alloc_psum_tensor("ppre0", [128, 512], F32)
    ppre1 = nc.alloc_psum_tensor("ppre1", [128, 512], F32)
    ppre = [ppre0, ppre1]
    # 4 banks: [buffer][half] — a matmul output must stay inside one 2KB bank
    pproj = [[nc.alloc_psum_tensor(f"pproj{b}{h}", [128, 512], F32)
              for h in range(2)] for b in range(2)]

    s_w = nc.alloc_semaphore("s_w")       # weight DMAs (inc 16 each, 3 total)
    s_dx = nc.alloc_semaphore("s_dx")     # x group DMAs
    if X_INT8:
        s_cvx = nc.alloc_semaphore("s_cvx")  # x int8 -> bf16 group converts
    s_zpre = nc.alloc_semaphore("s_zpre")
    s_pre = nc.alloc_semaphore("s_pre")
    s_z = nc.alloc_semaphore("s_z")
    s_za = nc.alloc_semaphore("s_za")
    s_u = nc.alloc_semaphore("s_u")
    s_h = nc.alloc_semaphore("s_h")
    s_c1 = nc.alloc_semaphore("s_c1")
    s_c2 = nc.alloc_semaphore("s_c2")
    s_c3 = nc.alloc_semaphore("s_c3")
    s_c4 = nc.alloc_semaphore("s_c4")
    s_hist = nc.alloc_semaphore("s_hist")
    s_zproj = nc.alloc_semaphore("s_zproj")
    s_proj = nc.alloc_semaphore("s_proj")
    s_out0 = nc.alloc_semaphore("s_out0")
    s_out1 = nc.alloc_semaphore("s_out1")
    s_out = [s_out0, s_out1]

    def whs_tile(k, m):
        off = (k * MT + m) * 128
        return whs[:, off:off + 128]

    def wxs_tile(k, m):
        off = (k * MT + m) * 128
        return wxs[:, off:off + 128]

    # hist aliases wxs: [128, chunk(8), t] bf16 (chunk-major)
    hist_r = wxs.ap().rearrange("p (c t) -> p c t", c=K)
    a_r = a_sb.ap().rearrange("p (t m) -> p t m", m=MT)
    xbf_r = xbf.ap().rearrange("p (s c t) -> p s c t", s=2, c=K)
    xdma = xi8 if X_INT8 else xbf
    xdma_r = xdma.ap().rearrange("p (s c t) -> p s c t", s=2, c=K)
    hbf_3 = hbf.ap().rearrange("p (c o) -> p c o", o=1)

    with nc.Block() as block:

        @block.sync
        def _(sync):
            # phase 0: weights straight into SBUF (single DMA each)
            sync.dma_start(
                whs.ap().rearrange("p (k n) -> p k n", k=K),
                wh.ap().rearrange("(k p) n -> p k n", p=128),
            ).then_inc(s_w, 16)
            sync.dma_start(
                wxs.ap().rearrange("p (k n) -> p k n", k=K),
                wx.ap().rearrange("(k p) n -> p k n", p=128),
            ).then_inc(s_w, 16)
            sync.dma_start(
                wps.ap().rearrange("p (k n) -> p k n", k=K),
                wp.ap().rearrange("(k p) n -> p k n", p=128),
            ).then_inc(s_w, 16)
            # phase 1: x group loads (double-buffered slots)
            for g in range(ng):
                if g >= 2:
                    if X_INT8:
                        sync.wait_ge(s_cvx, g - 1)
                    else:
                        sync.wait_ge(s_zpre, MT * (g - 1))
                sync.dma_start(
                    xdma_r[:, g % 2, :, :],
                    xt[:, g * 512:(g + 1) * 512].rearrange(
                        "(c p) t -> p c t", p=128),
                ).then_inc(s_dx, 16)
            # phase 3: y tile stores
            for idx in range(tb_total):
                sync.wait_ge(s_proj, idx + 1)
                sync.dma_start(
                    y[idx * 128:(idx + 1) * 128, :],
                    ysb[idx % 2][:],
                ).then_inc(s_out[idx % 2], 16)
            sync.wait_ge(s_out[0], 16 * ((tb_total + 1) // 2))
            sync.wait_ge(s_out[1], 16 * (tb_total // 2))

        @block.vector
        def _(vector):
            # phase 1: x int8 -> bf16 group converts (vector is idle here)
            if X_INT8:
                for g in range(ng):
                    vector.wait_ge(s_dx, 16 * (g + 1))
                    if g >= 2:
                        vector.wait_ge(s_zpre, MT * (g - 1))
                    gb = (g % 2) * K * 512
                    vector.tensor_copy(
                        xbf[:, gb:gb + K * 512], xi8[:, gb:gb + K * 512],
                    ).then_inc(s_cvx, 1)
            # phase 2 preamble
            vector.memset(h32[:], 0.0)
            vector.memset(hbf[:], 0.0).then_inc(s_h, 1)
            for i in range(t_total):
                cnt = i + 1
                za_slot = za_sb[:, (i % 2) * 16:(i % 2) * 16 + 16]
                vector.wait_ge(s_z, cnt)
                vector.tensor_add(
                    za_slot, zps[:], a_sb[:, i * 16:(i + 1) * 16],
                ).then_inc(s_za, 1)
                vector.wait_ge(s_u, cnt)
                vector.wait_ge(s_hist, i)  # hist(t-1) done reading r_sb
                uf, ug = u_sb[:, 0:8], u_sb[:, 8:16]
                vector.tensor_sub(d_sb[:], h32[:], ug).then_inc(s_c1, 1)
                vector.tensor_add(q_sb[:], h32[:], ug).then_inc(s_c2, 1)
                vector.wait_ge(s_c1, cnt)
                vector.tensor_mul(p_sb[:], uf, d_sb[:]).then_inc(s_c3, 1)
                vector.wait_ge(s_c2, cnt)
                vector.wait_ge(s_c3, cnt)
                # hbf = p + q = 2*h in bf16; W_h is pre-halved on the host
                # so downstream matmuls see h exactly.
                vector.tensor_add(hbf[:], p_sb[:], q_sb[:]).then_inc(s_h, 1)
                vector.tensor_add(r_sb[:], p_sb[:], q_sb[:]).then_inc(s_c4, 1)
                vector.wait_ge(s_c4, cnt)
                vector.tensor_scalar_mul(h32[:], r_sb[:], 0.5)
                r_3 = r_sb.ap().rearrange("p (c o) -> p c o", o=1)
                vector.wait_ge(s_c4, cnt)
                vector.tensor_scalar_mul(
                    hist_r[:, :, i:i + 1], r_3, 0.5).then_inc(s_hist, 1)

        @block.scalar
        def _(scalar):
            # phase 1: psum -> a_sb copies
            for idx in range(ng * MT):
                g, m = idx // MT, idx % MT
                scalar.wait_ge(s_zpre, idx + 1)
                scalar.copy(
                    a_r[:, g * 512:(g + 1) * 512, m],
                    ppre[idx % 2][:],
                ).then_inc(s_pre, 1)
            # phase 2: the tanh
            for i in range(t_total):
                scalar.wait_ge(s_za, i + 1)
                zbase = (i % 2) * 16
                scalar.activation(
                    u_sb[:], za_sb[:, zbase:zbase + 16],
                    mybir.ActivationFunctionType.Tanh, scale=0.5,
                ).then_inc(s_u, 1)
            # phase 3: psum -> bf16 staging for the y stores
            for idx in range(tb_total):
                scalar.wait_ge(s_zproj, idx + 1)
                if idx >= 2:
                    scalar.wait_ge(s_out[idx % 2], 16 * ((idx - 2) // 2 + 1))
                scalar.copy(ysb[idx % 2][:, 0:512], pproj[idx % 2][0][:])
                scalar.copy(ysb[idx % 2][:, 512:1024], pproj[idx % 2][1][:]) \
                      .then_inc(s_proj, 1)

        @block.tensor
        def _(tensor):
            # phase 1: A = W_x^T x, 512 t at a time
            for g in range(ng):
                for m in range(MT):
                    idx = g * MT + m
                    if m == 0:
                        if X_INT8:
                            tensor.wait_ge(s_cvx, g + 1)
                        else:
                            tensor.wait_ge(s_dx, 16 * (g + 1))
                        if g == 0:
                            tensor.wait_ge(s_w, 48)  # all weight DMAs done
                    if idx >= 2:
                        tensor.wait_ge(s_pre, idx - 1)
                    for k in range(K):
                        mm = tensor.matmul(
                            ppre[idx % 2][:],
                            wxs_tile(k, m),
                            xbf_r[:, g % 2, k, :],
                            start=(k == 0), stop=(k == K - 1),
                        )
                    mm.then_inc(s_zpre, 1)
            tensor.wait_ge(s_pre, ng * MT)
            # phase 2: z_h = (0.5 W_h)^T (2 h)
            for i in range(t_total):
                tensor.wait_ge(s_h, i + 1)
                for m in range(MT):
                    for k in range(K):
                        mm = tensor.matmul(
                            zps[:, m:m + 1],
                            whs_tile(k, m),
                            hbf[:, k:k + 1],
                            start=(k == 0), stop=(k == K - 1),
                        )
                mm.then_inc(s_z, 1)
            # phase 3: y^T blocks: stationary = hist tile, moving = W_proj
            tensor.wait_ge(s_h, t_total + 1)
            tensor.wait_ge(s_hist, t_total)
            tensor.wait_ge(s_w, 48)  # wps loaded
            for idx in range(tb_total):
                if idx >= 2:
                    tensor.wait_ge(s_proj, idx - 1)
                for h in range(2):
                    for j in range(K):
                        mm = tensor.matmul(
                            pproj[idx % 2][h][:],
                            hist_r[:, j, idx * 128:(idx + 1) * 128],
                            wps[:, j * C + h * 512:j * C + h * 512 + 512],
                            start=(j == 0), stop=(j == K - 1),
                        )
                mm.then_inc(s_zproj, 1)

    nc.compile()
    return nc


# ---------------------------------------------------------------- jax glue

_STATE = {}


def _get_state(t_total):
    if t_total in _STATE:
        return _STATE[t_total]
    import jax
    import jax.numpy as jnp
    from jax.sharding import Mesh, PartitionSpec as P, NamedSharding
    from jax.experimental.shard_map import shard_map
    from concourse import bass2jax
    from concourse.bass2jax import _bass_exec_p, partition_id_tensor

    bass2jax.install_neuronx_cc_hook()

    nc = build_nc(t_total)

    devs = jax.devices()[:B]
    mesh = Mesh(np.asarray(devs), ("core",))
    shard = NamedSharding(mesh, P("core"))

    # names in allocation order (must match the declarations in build_nc);
    # operands are inputs + output buffer + partition id, as in
    # bass2jax.run_bass_via_pjrt
    part_name = nc.partition_id_tensor.name if nc.partition_id_tensor else None
    in_names = ("xt", "wh", "wx", "wp", "y") + ((part_name,) if part_name else ())
    out_names = ("y",)
    out_avals = (jax.core.ShapedArray((t_total, C), jnp.bfloat16),)

    def _prep_x(x_loc):
        # x_loc [t_total, 1024] bf16 -> xt [1024, t_total]
        return x_loc.T

    prep_x = jax.jit(shard_map(
        _prep_x, mesh=mesh,
        in_specs=(P("core"),),
        out_specs=P("core"),
        check_rep=False,
    ))

    def _prep_w(w_loc):
        # w_loc [WPACK_ROWS//8, 2048] bf16
        w = jax.lax.all_gather(w_loc, "core", axis=0, tiled=True)
        whv = w[0:C]
        wxv = w[C:2 * C]
        wpv = w[2 * C:].reshape(C, C)
        return whv, wxv, wpv

    prep_w = jax.jit(shard_map(
        _prep_w, mesh=mesh,
        in_specs=(P("core"),),
        out_specs=(P("core"), P("core"), P("core")),
        check_rep=False,
    ))

    def _body(*args):
        operands = list(args)
        operands.append(partition_id_tensor())
        outs = _bass_exec_p.bind(
            *operands,
            out_avals=out_avals,
            in_names=in_names,
            out_names=out_names,
            lowering_input_output_aliases=(),
            sim_require_finite=True,
            sim_require_nnan=True,
            nc=nc,
        )
        return tuple(outs)

    kern = jax.jit(shard_map(
        _body, mesh=mesh,
        in_specs=(P("core"),) * 5,
        out_specs=(P("core"),),
        check_rep=False,
    ), keep_unused=True)

    # persistent dummy output operand (the kernel writes every element of y,
    # so its initial contents never matter; not donated so it is reusable)
    zjit = jax.jit(lambda: jnp.zeros((B * t_total, C), jnp.bfloat16),
                   out_shardings=shard)
    ydummy = jax.block_until_ready(zjit())

    from concurrent.futures import ThreadPoolExecutor
    st = {"nc": nc, "mesh": mesh, "shard": shard,
          "prep_x": prep_x, "prep_w": prep_w,
          "kern": kern, "ydummy": ydummy, "jax": jax,
          "pool": ThreadPoolExecutor(8),
          "wkey": None, "wdev": None}
    _STATE[t_total] = st
    return st


def _pack_weights(W_f, W_g, W_proj, x_scale=None):
    Cv = C
    wh_np = 0.5 * np.concatenate([W_f[Cv:], 2.0 * W_g[Cv:]], axis=1)
    wx_np = np.concatenate([W_f[:Cv], 2.0 * W_g[:Cv]], axis=1)
    if x_scale is not None:
        wx_np *= x_scale[:, None]  # fold the int8 x dequant scales
    pack = np.empty((WPACK_ROWS, 2 * Cv), dtype=BF)
    pack[0:Cv] = wh_np.astype(BF)
    pack[Cv:2 * Cv] = wx_np.astype(BF)
    pack[2 * Cv:] = W_proj.astype(BF).reshape(Cv // 2, 2 * Cv)
    return pack


def kernel(x, W_f, W_g, W_proj):
    t_total = x.shape[1]
    st = _get_state(t_total)
    jax = st["jax"]
    sh = st["shard"]
    th = t_total // 2

    # Weights are usually identical across calls: keep their device-resident,
    # all-gathered form cached, keyed on a content hash (cheap vs 10.5MB of
    # relay traffic + packing).
    import hashlib
    hsh = hashlib.blake2b(digest_size=16)
    W_f = np.ascontiguousarray(W_f, np.float32)
    W_g = np.ascontiguousarray(W_g, np.float32)
    W_proj = np.ascontiguousarray(W_proj, np.float32)
    for w in (W_f, W_g, W_proj):
        hsh.update(w)
    wkey = hsh.digest()
    if st["wkey"] != wkey:
        wpack = _pack_weights(W_f, W_g, W_proj)
        wd = jax.device_put(wpack, sh)
        st["wdev"] = jax.block_until_ready(st["prep_w"](wd))
        st["wkey"] = wkey
    whv, wxv, wpv = st["wdev"]

    # Single contiguous cast + one device_put: on this 1-core host, splitting
    # transfers to overlap with numpy work regresses (the relay's client pump
    # and numpy contend for the single CPU), so keep it sequential.
    xg = np.ascontiguousarray(x, dtype=np.float32).astype(BF) \
           .reshape(B * t_total, C)
    xd = jax.device_put(xg, sh)

    xt = st["prep_x"](xd)
    (y,) = st["kern"](xt, whv, wxv, wpv, st["ydummy"])

    # Fetch y shard-by-shard (the relay serializes anyway) and overlap the
    # bf16 -> f32 conversion of shard i with the fetch of shard i+1.
    out = np.empty((B, t_total, C), dtype=np.float32)
    shards = y.addressable_shards
    futs = [(s.index[0].start // t_total, st["pool"].submit(np.asarray, s.data))
            for s in shards]
    for core, fut in futs:
        out[core] = fut.result()
    return out
